# revision 57
# baseline (speedup 1.0000x reference)
"""Trainium2 Bass kernel for nn_DRNN (tree double-LSTM decoder + logits/log_softmax).

Strategy:
  - Pure data parallel: batch B=128 sharded 16 rows/core over 8 cores.
  - The T=40 recurrence is restructured:
      * ancestral LSTM: nodes processed by tree depth (max 11 levels for this
        data) with father h/c gathered via indirect DMA from a DRAM state
        buffer; x-side projections (embed @ wih.T + biases) are hoisted out of
        the loop and computed batched.
      * fraternal (sibling) LSTM: resets every 3 steps, so it collapses to a
        constant state + 2 batched rounds over 13 chains x 16 rows.
  - pred head and the big logits matmul + log_softmax run batched over all
    640 (b, t) rows per core; logit_w streamed in bf16; everything else fp32r.
"""

import sys

sys.path.insert(0, "/opt/trn_rl_repo")

import numpy as np
import ml_dtypes

import concourse.bass as bass
import concourse.bacc as bacc
import concourse.tile as tile
from concourse import mybir
from concourse import bass_utils
from concourse.masks import make_identity

F32 = mybir.dt.float32
F32R = mybir.dt.float32r
BF16 = mybir.dt.bfloat16
I32 = mybir.dt.int32
AF = mybir.ActivationFunctionType
OP = mybir.AluOpType

B, T, E, H, V, FC = 128, 40, 512, 512, 10000, 2048
NC_, BC = 8, 16          # cores, batch per core
NR = BC * T              # 640 rows per core
G = 4 * H                # 2048 gate dim
NV = 20                  # logits column chunks
VC = V // NV             # 500 cols per chunk
DUMP = NR                # dump row index in HC/HF

LAST_RESULTS = None
LAST_EXEC_NS = None
SKIP_PRED = False
SKIP_LOGITS = False


def _levels(fa):
    L = np.zeros((B, T), dtype=np.int32)
    rows = np.arange(B)
    for i in range(1, T):
        L[:, i] = 1 + L[rows, fa[:, i]]
    return L


def _chunks(n):
    out = []
    o = 0
    while o < n:
        out.append((o, min(128, n - o)))
        o += 128
    return out


def _build(NL, OL, XPAD, MCH_A):
    """Build the (SPMD-common) bass program. NL: common level sizes."""
    nc = bacc.Bacc("TRN2", target_bir_lowering=False, debug=True)

    dt_in = {}

    def din(name, shape, dt):
        t = nc.dram_tensor(name, list(shape), dt, kind="ExternalInput")
        dt_in[name] = t
        return t

    # pieces of the level schedule: (level, global_off, count)
    pieces = []
    for l in range(len(NL)):
        for (o, c) in _chunks(NL[l]):
            pieces.append((l + 1, OL[l] + o, c))
    NP = len(pieces)

    emb_a = din("emb_a", [4, 128, MCH_A * 128], F32R)
    emb_f = din("emb_f", [4, 128, 512], F32R)
    fcT = din("fcT", [16, 128, BC], F32R)
    fc_wT = din("fc_wT", [16, 128, H], F32R)
    wih_a = din("wih_a", [4, 128, G], F32R)
    wih_f = din("wih_f", [4, 128, G], F32R)
    whh_a = din("whh_a", [4, 128, G], F32R)
    whh_f = din("whh_f", [4, 128, G], F32R)
    pred_wT = din("pred_wT", [8, 128, H], F32R)
    pred_bT = din("pred_bT", [4, 128, 1], F32)
    lwT = din("lwT", [4, 128, V], BF16)
    bias_a = din("bias_a", [1, G], F32R)
    bias_f = din("bias_f", [1, G], F32R)
    fc_bT = din("fc_bT", [4, 128, 1], F32)
    logit_b = din("logit_b", [1, V], F32R)
    ones = din("ones", [1, 128], F32R)
    gsa = din("gsa", [NP, 128, 1], I32)
    NLV = len(NL)
    KPREV = [1] + [len(_chunks(NL[l])) for l in range(NLV - 1)]  # prev-level pieces
    NLP = [-(-n // 4) * 4 for n in NL]  # fp32r matmuls need even moving dim
    sels = [din(f"sel_{l + 1}", [KPREV[l], 128, NLP[l]], F32R) for l in range(NLV)]
    gsf1 = din("gsf1", [2, 128, 1], I32)
    gsf2 = din("gsf2", [2, 128, 1], I32)
    gshf0 = din("gshf0", [2, 128, 1], I32)

    OUT = nc.dram_tensor("OUT", [NR, V], F32, kind="ExternalOutput")

    with tile.TileContext(nc) as tc:
        with tc.tile_pool(name="p0", bufs=1) as p0, \
             tc.tile_pool(name="dram", bufs=1, space="DRAM") as pd, \
             tc.tile_pool(name="psg", bufs=6, space="PSUM") as psg, \
             tc.tile_pool(name="pst", bufs=2, space="PSUM") as pst:

            HC = pd.tile([NR + 1, H], F32)         # h ancestral, natural rows
            HF = pd.tile([NR + 1, H], F32)         # h fraternal, natural rows
            XA = pd.tile([MCH_A * 128, G], F32)    # x-projection, level order
            XF = pd.tile([512, G], F32)            # x-projection, s order

            ident = p0.tile([128, 128], F32)
            make_identity(nc, ident[:])
            ones_t = p0.tile([1, 128], F32R)
            nc.sync.dma_start(ones_t[:], ones[:])
            bias_a_t = p0.tile([1, G], F32R)
            nc.sync.dma_start(bias_a_t[:], bias_a[:])
            bias_f_t = p0.tile([1, G], F32R)
            nc.sync.dma_start(bias_f_t[:], bias_f[:])
            fc_bT_t = p0.tile([128, 4, 1], F32)
            pred_bT_t = p0.tile([128, 4, 1], F32)
            for q in range(4):
                nc.sync.dma_start(fc_bT_t[:, q, :], fc_bT[q])
                nc.sync.dma_start(pred_bT_t[:, q, :], pred_bT[q])

            gsa_t = p0.tile([128, NP, 1], I32)
            for p in range(NP):
                nc.sync.dma_start(gsa_t[:, p, :], gsa[p])
            gsf1_t = p0.tile([128, 2, 1], I32)
            gsf2_t = p0.tile([128, 2, 1], I32)
            gshf0_t = p0.tile([128, 2, 1], I32)
            for p in range(2):
                nc.sync.dma_start(gsf1_t[:, p, :], gsf1[p])
                nc.sync.dma_start(gsf2_t[:, p, :], gsf2[p])
                nc.sync.dma_start(gshf0_t[:, p, :], gshf0[p])

            # zero-init state buffers (also covers the dump row)
            z_t = p0.tile([128, 2 * H], F32)
            nc.vector.memset(z_t[:], 0.0)
            for o in range(0, NR + 1, 128):
                c = min(128, NR + 1 - o)
                nc.sync.dma_start(HC[o:o + c, :], z_t[:c, :H])
                nc.sync.dma_start(HF[o:o + c, :], z_t[:c, :H])

            # persistent mid-size tiles
            xa0T = p0.tile([128, 4, BC], F32R)     # transposed fc projection
            hf0 = p0.tile([128, H], F32)           # broadcast const states
            cf0 = p0.tile([128, H], F32)
            hf0T = p0.tile([128, 4, 1], F32R)
            w0f = p0.tile([1, G], F32R)
            outT = p0.tile([128, 4, NR], BF16)     # pred output transposed
            hr0 = p0.tile([128, 2 * H], F32R)      # level-0 h|c, rounded for sel-matmul

            def elementwise(t_sb, c_in, hc2, pc, gact, tc2):
                """gates t_sb [pc, G] (i f g o), c_in [pc, H] or None -> hc2 [pc, h|c]"""
                nc.scalar.activation(gact[:pc, 0:2 * H], t_sb[:pc, 0:2 * H], AF.Sigmoid)
                nc.scalar.activation(gact[:pc, 2 * H:3 * H], t_sb[:pc, 2 * H:3 * H], AF.Tanh)
                nc.scalar.activation(gact[:pc, 3 * H:4 * H], t_sb[:pc, 3 * H:4 * H], AF.Sigmoid)
                # c2 = f*c + i*g~   (written to hc2[:, H:2H])
                if c_in is not None:
                    nc.vector.tensor_tensor(out=hc2[:pc, H:2 * H], in0=gact[:pc, H:2 * H],
                                            in1=c_in[:pc, :], op=OP.mult)
                    nc.vector.tensor_tensor(out=tc2[:pc, :], in0=gact[:pc, 0:H],
                                            in1=gact[:pc, 2 * H:3 * H], op=OP.mult)
                    nc.vector.tensor_tensor(out=hc2[:pc, H:2 * H], in0=hc2[:pc, H:2 * H],
                                            in1=tc2[:pc, :], op=OP.add)
                else:
                    nc.vector.tensor_tensor(out=hc2[:pc, H:2 * H], in0=gact[:pc, 0:H],
                                            in1=gact[:pc, 2 * H:3 * H], op=OP.mult)
                # h2 = o * tanh(c2)  (written to hc2[:, 0:H])
                nc.scalar.activation(tc2[:pc, :], hc2[:pc, H:2 * H], AF.Tanh)
                nc.vector.tensor_tensor(out=hc2[:pc, 0:H], in0=gact[:pc, 3 * H:4 * H],
                                        in1=tc2[:pc, :], op=OP.mult)

            # ---------------- fc path: x_a0T = fc_w @ fc_feats.T  ----------------
            with tc.tile_pool(name="pfc", bufs=1) as pfc:
                fcT_t = pfc.tile([128, 16, BC], F32R)
                fc_wT_t = pfc.tile([128, 16, H], F32R)
                nc.sync.dma_start(fcT_t[:], fcT[:].rearrange("q p n -> p q n"))
                nc.sync.dma_start(fc_wT_t[:], fc_wT[:].rearrange("q p n -> p q n"))
                for mm in range(4):
                    pp = pst.tile([128, BC], F32, space="PSUM", tag="ptr2")
                    for q in range(16):
                        nc.tensor.matmul(pp[:, :], fc_wT_t[:, q, mm * 128:(mm + 1) * 128],
                                         fcT_t[:, q, :], start=(q == 0), stop=(q == 15))
                    # x_a0T chunk [128, BC] + fc_b per-partition bias
                    nc.scalar.activation(xa0T[:, mm, :], pp[:, :], AF.Identity,
                                         bias=fc_bT_t[:, mm, :])

            # ---------------- projections XA / XF + bias ----------------
            with tc.tile_pool(name="pproj", bufs=1) as ppj, \
                 tc.tile_pool(name="pw1", bufs=2) as pw1:
                emb_a_t = ppj.tile([128, 4, MCH_A * 128], F32R)
                emb_f_t = ppj.tile([128, 4, 512], F32R)
                wih_a_t = ppj.tile([128, 4, G], F32R)
                wih_f_t = ppj.tile([128, 4, G], F32R)
                nc.sync.dma_start(emb_a_t[:], emb_a[:].rearrange("q p n -> p q n"))
                nc.sync.dma_start(emb_f_t[:], emb_f[:].rearrange("q p n -> p q n"))
                nc.sync.dma_start(wih_a_t[:], wih_a[:].rearrange("q p n -> p q n"))
                nc.sync.dma_start(wih_f_t[:], wih_f[:].rearrange("q p n -> p q n"))
                for (src, w, bias_row, dst, nm) in (
                        (emb_a_t, wih_a_t, bias_a_t, XA, MCH_A),
                        (emb_f_t, wih_f_t, bias_f_t, XF, 4)):
                    for m in range(nm):
                        for n in range(4):
                            pg = psg.tile([128, 512], F32, space="PSUM", tag="pg")
                            for q in range(4):
                                nc.tensor.matmul(pg[:, :], src[:, q, m * 128:(m + 1) * 128],
                                                 w[:, q, n * 512:(n + 1) * 512],
                                                 start=(q == 0), stop=False)
                            nc.tensor.matmul(pg[:, :], ones_t[:1, :128],
                                             bias_row[:1, n * 512:(n + 1) * 512],
                                             start=False, stop=True)
                            xc = pw1.tile([128, 512], F32, tag="xc")
                            if n % 2 == 0:
                                nc.vector.tensor_copy(xc[:, :], pg[:, :])
                            else:
                                nc.scalar.copy(xc[:, :], pg[:, :])
                            nc.sync.dma_start(dst[m * 128:(m + 1) * 128, n * 512:(n + 1) * 512], xc[:, :])

                # ---------------- level 0 (fc input, zero state) ----------------
                t0_sb = pw1.tile([BC, G], F32, tag="tsb")
                for n in range(4):
                    pg = psg.tile([128, 512], F32, space="PSUM", tag="pg")
                    for q in range(4):
                        nc.tensor.matmul(pg[:BC, :], xa0T[:, q, :],
                                         wih_a_t[:, q, n * 512:(n + 1) * 512],
                                         start=(q == 0), stop=False)
                    nc.tensor.matmul(pg[:BC, :], ones_t[:1, :BC],
                                     bias_a_t[:1, n * 512:(n + 1) * 512],
                                     start=False, stop=True)
                    nc.vector.tensor_copy(t0_sb[:, n * 512:(n + 1) * 512], pg[:BC, :])
                gact0 = pw1.tile([BC, G], F32, tag="gact")
                hc20 = pw1.tile([BC, 2 * H], F32, tag="hc2")
                tc20 = pw1.tile([BC, H], F32, tag="tc2")
                elementwise(t0_sb, None, hc20, BC, gact0, tc20)
                nc.vector.tensor_copy(hr0[:BC, :], hc20[:, :])
                nc.sync.dma_start(
                    HC[0:NR, :].rearrange("(b t) d -> b t d", t=T)[:, 0, :], hc20[:, 0:H])

            # ---------------- constant fraternal state hf0/cf0, w0f ----------------
            with tc.tile_pool(name="pcst", bufs=1) as pcs:
                gbf = pcs.tile([128, G], F32)
                for n in range(4):
                    pg = psg.tile([128, 512], F32, space="PSUM", tag="pg")
                    nc.tensor.matmul(pg[:, :], ones_t[:1, :128],
                                     bias_f_t[:1, n * 512:(n + 1) * 512], start=True, stop=True)
                    nc.vector.tensor_copy(gbf[:, n * 512:(n + 1) * 512], pg[:, :])
                gactc = pcs.tile([128, G], F32)
                nc.scalar.activation(gactc[:, 0:2 * H], gbf[:, 0:2 * H], AF.Sigmoid)
                nc.scalar.activation(gactc[:, 2 * H:3 * H], gbf[:, 2 * H:3 * H], AF.Tanh)
                nc.scalar.activation(gactc[:, 3 * H:4 * H], gbf[:, 3 * H:4 * H], AF.Sigmoid)
                nc.vector.tensor_tensor(out=cf0[:, :], in0=gactc[:, 0:H],
                                        in1=gactc[:, 2 * H:3 * H], op=OP.mult)
                tcf0 = pcs.tile([128, H], F32)
                nc.scalar.activation(tcf0[:, :], cf0[:, :], AF.Tanh)
                nc.vector.tensor_tensor(out=hf0[:, :], in0=gactc[:, 3 * H:4 * H],
                                        in1=tcf0[:, :], op=OP.mult)
                # hf0T [H, 1] via 4 transposes of hf0[0:1, :]
                for q in range(4):
                    pt = pst.tile([128, 128], F32, space="PSUM", tag="ptr2")
                    nc.tensor.transpose(pt[:, :1], hf0[0:1, q * 128:(q + 1) * 128], ident[:1, :1])
                    nc.vector.tensor_copy(hf0T[:, q, :], pt[:, :1])
                # scatter hf0 rows to HF (i=0 and reset steps)
                nc.gpsimd.indirect_dma_start(
                    out=HF[:, :], out_offset=bass.IndirectOffsetOnAxis(ap=gshf0_t[:, 0, :], axis=0),
                    in_=hf0[:128, :], in_offset=None)
                nc.gpsimd.indirect_dma_start(
                    out=HF[:, :], out_offset=bass.IndirectOffsetOnAxis(ap=gshf0_t[:96, 1, :], axis=0),
                    in_=hf0[:96, :], in_offset=None)

            # ---------------- ancestral levels + fraternal chains ----------------
            with tc.tile_pool(name="prec", bufs=1) as prc, \
                 tc.tile_pool(name="pw2", bufs=2) as pw2:
                whh_a_t = prc.tile([128, 4, G], F32R)
                whh_f_t = prc.tile([128, 4, G], F32R)
                nc.sync.dma_start(whh_a_t[:], whh_a[:].rearrange("q p n -> p q n"))
                nc.sync.dma_start(whh_f_t[:], whh_f[:].rearrange("q p n -> p q n"))

                # w0f = hf0 @ whh_f.T  -> [1, G]
                for n in range(4):
                    pg = psg.tile([128, 512], F32, space="PSUM", tag="pg")
                    for q in range(4):
                        nc.tensor.matmul(pg[:1, :], hf0T[:, q, :],
                                         whh_f_t[:, q, n * 512:(n + 1) * 512],
                                         start=(q == 0), stop=(q == 3))
                    nc.vector.tensor_copy(w0f[:1, n * 512:(n + 1) * 512], pg[:1, :])

                def lstm_round(pc, haT, xrow_src, c_in, whh_t, extra_bias_row, hc2=None):
                    """one batched LSTM round: returns hc2 tile [pc, 2H]."""
                    t_sb = pw2.tile([128, G], F32, tag="tsb2")
                    for n in range(4):
                        pg = psg.tile([128, 512], F32, space="PSUM", tag="pg")
                        if haT is not None:
                            for q in range(4):
                                nc.tensor.matmul(pg[:pc, :], haT[:, q, :pc],
                                                 whh_t[:, q, n * 512:(n + 1) * 512],
                                                 start=(q == 0), stop=(extra_bias_row is None and q == 3))
                        if extra_bias_row is not None:
                            nc.tensor.matmul(pg[:pc, :], ones_t[:1, :pc],
                                             extra_bias_row[:1, n * 512:(n + 1) * 512],
                                             start=(haT is None), stop=True)
                        nc.vector.tensor_tensor(out=t_sb[:pc, n * 512:(n + 1) * 512],
                                                in0=pg[:pc, :],
                                                in1=xrow_src[:pc, n * 512:(n + 1) * 512], op=OP.add)
                    gact = pw2.tile([128, G], F32, tag="gact2")
                    if hc2 is None:
                        hc2 = pw2.tile([128, 2 * H], F32, tag="hc22")
                    tc2 = pw2.tile([128, H], F32, tag="tc22")
                    elementwise(t_sb, c_in, hc2, pc, gact, tc2)
                    return hc2

                def transpose_h(src, pc, tag):
                    """src [pc, H] -> haT tile [128, 4, pc] (f32r)"""
                    haT = pw2.tile([128, 4, 128], F32R, tag=tag)
                    for q in range(4):
                        pt = pst.tile([128, 128], F32, space="PSUM", tag="ptr2")
                        nc.tensor.transpose(pt[:, :pc], src[:pc, q * 128:(q + 1) * 128],
                                            ident[:pc, :pc])
                        nc.vector.tensor_copy(haT[:, q, :pc], pt[:, :pc])
                    return haT

                # fraternal rounds, emitted interleaved with ancestral levels so
                # the scheduler can fill PE gather-stalls with independent work
                hf1 = []

                def frat_s1(j, o, c):
                    xf_t = pw2.tile([128, G], F32, tag="xat", name=f"xf1_{j}")
                    nc.sync.dma_start(xf_t[:c, :], XF[o:o + c, :])
                    keep = prc.tile([128, 2 * H], F32, tag=f"hf1_{j}")
                    hc2 = lstm_round(c, None, xf_t, cf0, whh_f_t, w0f, hc2=keep)
                    nc.gpsimd.indirect_dma_start(
                        out=HF[:, :], out_offset=bass.IndirectOffsetOnAxis(ap=gsf1_t[:c, j, :], axis=0),
                        in_=hc2[:c, 0:H], in_offset=None)
                    hf1.append(hc2)

                def frat_s2(j, o, c):
                    xf_t = pw2.tile([128, G], F32, tag="xat", name=f"xf2_{j}")
                    nc.sync.dma_start(xf_t[:c, :], XF[256 + o:256 + o + c, :])
                    hfT = transpose_h(hf1[j], c, "haT")
                    hc2 = lstm_round(c, hfT, xf_t, hf1[j][:, H:2 * H], whh_f_t, None)
                    nc.gpsimd.indirect_dma_start(
                        out=HF[:, :], out_offset=bass.IndirectOffsetOnAxis(ap=gsf2_t[:c, j, :], axis=0),
                        in_=hc2[:c, 0:H], in_offset=None)

                frat = [(frat_s1, j, o, c) for j, (o, c) in enumerate(_chunks(208))] + \
                       [(frat_s2, j, o, c) for j, (o, c) in enumerate(_chunks(208))]

                # ancestral levels: father h/c of level l live in level l-1's
                # SBUF output; gather via host-baked 0/1 selection matmuls.
                # haT comes out directly transposed (lhsT = h_prev straight).
                prev_pieces = [(hr0, BC)]
                pidx = 0
                for l in range(1, len(NL) + 1):
                    if l in (2, 3, 4, 5) and frat:
                        fn, j, o, c = frat.pop(0)
                        fn(j, o, c)
                    sel_t = pw2.tile([128, len(prev_pieces), NLP[l - 1]], F32R,
                                     tag="sel", name=f"sel_t{l}")
                    nc.sync.dma_start(sel_t[:], sels[l - 1][:].rearrange("k p n -> p k n"))
                    new_pieces = []
                    for (o_lvl, pc) in _chunks(NL[l - 1]):
                        po = int(OL[l - 1]) + o_lvl
                        pcg = max(pc, 2)
                        xa_t = pw2.tile([128, G], F32, tag="xat")
                        nc.sync.dma_start(xa_t[:pc, :], XA[po:po + pc, :])
                        # gather haT [512, pc] and c [pc, 512] from prev level
                        haT = pw2.tile([128, 4, 128], F32R, tag="haT")
                        pcp = min(-(-pc // 4) * 4, 128)
                        for mm in range(4):
                            ph = pst.tile([128, 128], F32, space="PSUM", tag="ptr2")
                            for kj, (hrp, pck) in enumerate(prev_pieces):
                                nc.tensor.matmul(ph[:, :pcp], hrp[:pck, mm * 128:(mm + 1) * 128],
                                                 sel_t[:pck, kj, o_lvl:o_lvl + pcp],
                                                 start=(kj == 0), stop=(kj == len(prev_pieces) - 1))
                            nc.vector.tensor_copy(haT[:, mm, :pc], ph[:, :pc])
                        pcg_ps = psg.tile([128, 512], F32, space="PSUM", tag="pg")
                        for kj, (hrp, pck) in enumerate(prev_pieces):
                            nc.tensor.matmul(pcg_ps[:pc, :], sel_t[:pck, kj, o_lvl:o_lvl + pc],
                                             hrp[:pck, H:2 * H],
                                             start=(kj == 0), stop=(kj == len(prev_pieces) - 1))
                        cg = pw2.tile([128, H], F32, tag="cg")
                        nc.vector.tensor_copy(cg[:pc, :], pcg_ps[:pc, :])
                        hc2_pre = None
                        if pcg > pc:
                            hc2_pre = pw2.tile([128, 2 * H], F32, tag="hc22", name=f"hc2p_{pidx}")
                            nc.vector.memset(hc2_pre[:pcg, :], 0.0)
                        hc2 = lstm_round(pc, haT, xa_t, cg, whh_a_t, None, hc2=hc2_pre)
                        # f32r copy for next level's selection matmuls
                        hr = prc.tile([128, 2 * H], F32R, tag=f"hr_{l % 2}_{len(new_pieces)}")
                        nc.vector.tensor_copy(hr[:pc, :], hc2[:pc, :])
                        new_pieces.append((hr, pc))
                        # scatter h to natural rows for the pred head
                        nc.gpsimd.indirect_dma_start(
                            out=HC[:, :], out_offset=bass.IndirectOffsetOnAxis(ap=gsa_t[:pcg, pidx, :], axis=0),
                            in_=hc2[:pcg, 0:H], in_offset=None)
                        pidx += 1
                    prev_pieces = new_pieces

                # any fraternal rounds not consumed by the interleave
                for fn, j, o, c in frat:
                    fn(j, o, c)

            # ---------------- pred head (transposed): outT = tanh(predW @ cat) ----------------
            if SKIP_PRED:
                return _fin(nc)
            with tc.tile_pool(name="ppred", bufs=1) as ppr, \
                 tc.tile_pool(name="pw3", bufs=3) as pw3:
                pred_wT_t = ppr.tile([128, 8, H], F32R)
                catT = ppr.tile([128, 8, NR], F32R)   # pred input transposed
                nc.sync.dma_start(pred_wT_t[:], pred_wT[:].rearrange("q p n -> p q n"))
                for m in range(5):
                    hA = pw3.tile([128, H], F32, tag="hA")
                    nc.sync.dma_start(hA[:, :], HC[m * 128:(m + 1) * 128, 0:H])
                    hFt = pw3.tile([128, H], F32, tag="hF")
                    nc.sync.dma_start(hFt[:, :], HF[m * 128:(m + 1) * 128, :])
                    for q in range(4):
                        pt = pst.tile([128, 128], F32, space="PSUM", tag="ptr2")
                        nc.tensor.transpose(pt[:, :], hA[:, q * 128:(q + 1) * 128], ident[:, :])
                        nc.vector.tensor_copy(catT[:, q, m * 128:(m + 1) * 128], pt[:, :])
                        pt2 = pst.tile([128, 128], F32, space="PSUM", tag="ptr2")
                        nc.tensor.transpose(pt2[:, :], hFt[:, q * 128:(q + 1) * 128], ident[:, :])
                        nc.vector.tensor_copy(catT[:, 4 + q, m * 128:(m + 1) * 128], pt2[:, :])
                for mm in range(4):
                    for (ns, nl) in ((0, 512), (512, 128)):
                        pg = psg.tile([128, 512], F32, space="PSUM", tag="pg")
                        for q in range(8):
                            nc.tensor.matmul(pg[:, :nl], pred_wT_t[:, q, mm * 128:(mm + 1) * 128],
                                             catT[:, q, ns:ns + nl], start=(q == 0), stop=(q == 7))
                        nc.scalar.activation(outT[:, mm, ns:ns + nl], pg[:, :nl], AF.Tanh,
                                             bias=pred_bT_t[:, mm, :])

            # ---------------- logits + log_softmax ----------------
            if SKIP_LOGITS:
                return _fin(nc)
            with tc.tile_pool(name="plg", bufs=1) as plg, \
                 tc.tile_pool(name="plw", bufs=4) as plw, \
                 tc.tile_pool(name="pls", bufs=2) as pls:
                lb_bcast = plg.tile([128, V], BF16)
                with tc.tile_pool(name="plb", bufs=1) as plb:
                    logit_b_t = plb.tile([1, V], BF16)
                    nc.gpsimd.dma_start(logit_b_t[:], logit_b[:].bitcast(F32))
                    ones_bf = plb.tile([1, 128], BF16)
                    nc.vector.memset(ones_bf[:], 1.0)
                    for n in range(NV):
                        pg = psg.tile([128, 512], F32, space="PSUM", tag="pg")
                        nc.tensor.matmul(pg[:, :VC], ones_bf[:1, :128],
                                         logit_b_t[:1, n * VC:(n + 1) * VC], start=True, stop=True)
                        nc.vector.tensor_copy(lb_bcast[:, n * VC:(n + 1) * VC], pg[:, :VC])
                sums = plg.tile([128, 5, NV], F32)
                lse = plg.tile([128, 5, 1], F32)
                lse2 = plg.tile([128, 5, 1], F32)
                for grp in ((0, 1, 2, 3, 4),):
                    lgs = {}
                    for m in grp:
                        lg_t = plg.tile([128, V], BF16, tag=f"lgs{m}", name=f"lgs_{m}")
                        lgs[m] = lg_t
                    for n in range(NV):
                        lw_t = plw.tile([128, 4, VC], BF16, tag="lw")
                        nc.sync.dma_start(lw_t[:], lwT[:, :, n * VC:(n + 1) * VC].rearrange("q p n -> p q n"))
                        for m in grp:
                            pg = psg.tile([128, 512], F32, space="PSUM", tag="pg")
                            for q in range(4):
                                nc.tensor.matmul(pg[:, :VC], outT[:, q, m * 128:(m + 1) * 128],
                                                 lw_t[:, q, :], start=(q == 0), stop=(q == 3))
                            nc.vector.tensor_tensor(out=lgs[m][:, n * VC:(n + 1) * VC],
                                                    in0=pg[:, :VC],
                                                    in1=lb_bcast[:, n * VC:(n + 1) * VC], op=OP.add)
                            esc = pls.tile([128, VC], BF16, tag="esc")
                            nc.scalar.activation(esc[:, :], lgs[m][:, n * VC:(n + 1) * VC],
                                                 AF.Exp, accum_out=sums[:, m, n:n + 1])
                    for m in grp:
                        nc.vector.tensor_reduce(out=lse[:, m, :], in_=sums[:, m, :],
                                                axis=mybir.AxisListType.X, op=OP.add)
                        nc.scalar.activation(lse2[:, m, :], lse[:, m, :], AF.Ln)
                        for n in range(NV):
                            oc = pls.tile([128, VC], F32, tag="oc")
                            nc.vector.tensor_scalar(out=oc[:, :], in0=lgs[m][:, n * VC:(n + 1) * VC],
                                                    scalar1=lse2[:, m, :1], scalar2=None,
                                                    op0=OP.subtract)
                            nc.sync.dma_start(OUT[m * 128:(m + 1) * 128, n * VC:(n + 1) * VC], oc[:, :])

    return _fin(nc)


def _fin(nc):
    nc.finalize()
    return nc


def _prep(word_idx, father_idx, fc_feats, embed, fc_w, fc_b,
          a_wih, a_whh, a_bih, a_bhh, f_wih, f_whh, f_bih, f_bhh,
          pred_w, pred_b, logit_w, logit_b):
    wi = np.asarray(word_idx).astype(np.int64)
    fa = np.asarray(father_idx).astype(np.int64)
    fc_feats = np.asarray(fc_feats, dtype=np.float32)
    embed = np.asarray(embed, dtype=np.float32)
    L = _levels(fa)
    Lmax = int(L.max())
    NL = []
    for l in range(1, Lmax + 1):
        NL.append(max(int((L[c * BC:(c + 1) * BC] == l).sum()) for c in range(NC_)))
    OL = np.concatenate([[0], np.cumsum(NL)]).astype(int)
    XPAD = int(OL[-1])
    MCH_A = -(-XPAD // 128)

    pieces = []
    for l in range(len(NL)):
        for (o, c) in _chunks(NL[l]):
            pieces.append((l + 1, int(OL[l]) + o, c))
    NP = len(pieces)

    embT = np.ascontiguousarray(embed.T)              # [E, V]
    wih_aT = np.ascontiguousarray(a_wih.T, dtype=np.float32).reshape(4, 128, G)
    wih_fT = np.ascontiguousarray(f_wih.T, dtype=np.float32).reshape(4, 128, G)
    whh_aT = np.ascontiguousarray(a_whh.T, dtype=np.float32).reshape(4, 128, G)
    whh_fT = np.ascontiguousarray(f_whh.T, dtype=np.float32).reshape(4, 128, G)
    fc_wT = np.ascontiguousarray(np.asarray(fc_w, np.float32).T).reshape(16, 128, H)
    pred_wT_ = np.ascontiguousarray(np.asarray(pred_w, np.float32).T).reshape(8, 128, H)
    pred_bT_ = np.asarray(pred_b, np.float32).reshape(4, 128, 1)
    lwT_ = np.ascontiguousarray(np.asarray(logit_w, np.float32).T.astype(ml_dtypes.bfloat16)).reshape(4, 128, V)
    bias_a_ = (np.asarray(a_bih, np.float32) + np.asarray(a_bhh, np.float32)).reshape(1, G)
    bias_f_ = (np.asarray(f_bih, np.float32) + np.asarray(f_bhh, np.float32)).reshape(1, G)
    logit_b_ = np.asarray(logit_b, np.float32).reshape(1, V)
    ones_ = np.ones((1, 128), np.float32)

    in_maps = []
    for c in range(NC_):
        gb0 = c * BC
        # ancestral node order: by (level, b, i)
        emb_a_ = np.zeros((4, 128, MCH_A * 128), np.float32)
        gsa_ = np.full((NP, 128, 1), DUMP, np.int32)
        sels_ = {}
        Lc = L[gb0:gb0 + BC]
        pos_prev = {(b, 0): b for b in range(BC)}
        for l in range(1, Lmax + 1):
            nodes = [(b, i) for b in range(BC) for i in range(1, T) if Lc[b, i] == l]
            kprev = 1 if l == 1 else len(_chunks(NL[l - 2]))
            sel = np.zeros((kprev, 128, -(-NL[l - 1] // 4) * 4), np.float32)
            pos_cur = {}
            for j, (b, i) in enumerate(nodes):
                p = int(OL[l - 1]) + j
                pos_cur[(b, i)] = j
                wa = wi[gb0 + b, fa[gb0 + b, i]]
                emb_a_[:, :, p] = embT[:, wa].reshape(4, 128)
                jp = pos_prev[(b, int(fa[gb0 + b, i]))]
                sel[jp // 128, jp % 128, j] = 1.0
                for pidx, (pl, po, pc) in enumerate(pieces):
                    if pl == l and po <= p < po + pc:
                        gsa_[pidx, p - po, 0] = b * T + i
                        break
            sels_[f"sel_{l}"] = sel
            pos_prev = pos_cur
        emb_f_ = np.zeros((4, 128, 512), np.float32)
        gsf1_ = np.full((2, 128, 1), DUMP, np.int32)
        gsf2_ = np.full((2, 128, 1), DUMP, np.int32)
        for b in range(BC):
            for k in range(13):
                p = b * 13 + k
                emb_f_[:, :, p] = embT[:, wi[gb0 + b, 3 * k + 1]].reshape(4, 128)
                emb_f_[:, :, 256 + p] = embT[:, wi[gb0 + b, 3 * k + 2]].reshape(4, 128)
                gsf1_[p // 128, p % 128, 0] = b * T + 3 * k + 2
                gsf2_[p // 128, p % 128, 0] = b * T + 3 * k + 3
        gshf0_ = np.full((2, 128, 1), DUMP, np.int32)
        hf0_rows = [b * T + i for b in range(BC) for i in ([0] + list(range(1, T, 3)))]
        for j, r in enumerate(hf0_rows):
            gshf0_[j // 128, j % 128, 0] = r
        fcT_ = np.ascontiguousarray(fc_feats[gb0:gb0 + BC].T).reshape(16, 128, BC)

        in_maps.append({
            "emb_a": emb_a_, "emb_f": emb_f_, "fcT": fcT_, "fc_wT": fc_wT,
            "wih_a": wih_aT, "wih_f": wih_fT, "whh_a": whh_aT, "whh_f": whh_fT,
            "pred_wT": pred_wT_, "pred_bT": pred_bT_, "lwT": lwT_,
            "bias_a": bias_a_, "bias_f": bias_f_,
            "fc_bT": np.asarray(fc_b, np.float32).reshape(4, 128, 1),
            "logit_b": logit_b_, "ones": ones_,
            "gsa": gsa_, "gsf1": gsf1_, "gsf2": gsf2_, "gshf0": gshf0_,
            **sels_,
        })
    return in_maps, NL, OL, XPAD, MCH_A


def kernel(**inputs):
    global LAST_RESULTS, LAST_EXEC_NS
    in_maps, NL, OL, XPAD, MCH_A = _prep(**inputs)
    nc = _build(NL, OL, XPAD, MCH_A)
    res = bass_utils.run_bass_kernel_spmd(nc, in_maps, core_ids=list(range(NC_)))
    LAST_RESULTS = res
    LAST_EXEC_NS = res.exec_time_ns
    outs = [res.results[c]["OUT"].reshape(BC, T, V) for c in range(NC_)]
    return np.concatenate(outs, axis=0).astype(np.float32)


# ---------------------------------------------------------------------------
# Timing helper (not used by grading): the axon NTFF profile hook is absent in
# this container, so estimate device exec time by pairing executes of this
# kernel against a trivial kernel with device-resident inputs; the axon
# dispatch overhead (~100ms, high variance) cancels in the paired difference.
def _make_runner(nc, in_maps, n_cores=NC_):
    import jax
    from jax.sharding import Mesh, PartitionSpec, NamedSharding
    from concourse import bass2jax

    bass2jax.install_neuronx_cc_hook()
    if nc.dbg_addr is not None:
        in_maps = [{**m, nc.dbg_addr.name: np.zeros((1, 2), np.uint32)} for m in in_maps]
    partition_name = nc.partition_id_tensor.name if nc.partition_id_tensor else None
    in_names, out_names, out_avals, zero_outs = [], [], [], []
    for alloc in nc.m.functions[0].allocations:
        if not isinstance(alloc, mybir.MemoryLocationSet):
            continue
        name = alloc.memorylocations[0].name
        if alloc.kind == "ExternalInput":
            if name != partition_name:
                in_names.append(name)
        elif alloc.kind == "ExternalOutput":
            out_names.append(name)
            shape = tuple(alloc.tensor_shape)
            dtype = mybir.dt.np(alloc.dtype)
            out_avals.append(jax.core.ShapedArray(shape, dtype))
            zero_outs.append(np.zeros(shape, dtype))
    n_params = len(in_names)
    all_in_names = list(in_names) + list(out_names)
    if partition_name is not None:
        all_in_names.append(partition_name)

    def _body(*args):
        operands = list(args)
        if partition_name is not None:
            operands.append(bass2jax.partition_id_tensor())
        outs = bass2jax._bass_exec_p.bind(
            *operands, out_avals=tuple(out_avals), in_names=tuple(all_in_names),
            out_names=tuple(out_names), lowering_input_output_aliases=(),
            sim_require_finite=True, sim_require_nnan=True, nc=nc)
        return tuple(outs)

    devices = jax.devices()[:n_cores]
    mesh = Mesh(np.asarray(devices), ("core",))
    in_specs = (PartitionSpec("core"),) * (n_params + len(out_names))
    out_specs = (PartitionSpec("core"),) * len(out_names)
    sharded = jax.jit(
        jax.shard_map(_body, mesh=mesh, in_specs=in_specs, out_specs=out_specs,
                      check_vma=False), keep_unused=True)
    concat_in = [np.concatenate([np.asarray(in_maps[c][nm]) for c in range(n_cores)], axis=0)
                 for nm in in_names]
    concat_zeros = [np.zeros((n_cores * z.shape[0], *z.shape[1:]), z.dtype) for z in zero_outs]
    sh = NamedSharding(mesh, PartitionSpec("core"))
    dev_args = [jax.device_put(x, sh) for x in concat_in + concat_zeros]
    return sharded, dev_args


def _trivial_nc():
    nc = bacc.Bacc("TRN2", target_bir_lowering=False, debug=True)
    x = nc.dram_tensor("x", [128, 512], F32, kind="ExternalInput")
    y = nc.dram_tensor("y", [128, 512], F32, kind="ExternalOutput")
    with tile.TileContext(nc) as tc:
        with tc.tile_pool(name="sb", bufs=2) as pool:
            t = pool.tile([128, 512], F32)
            nc.sync.dma_start(t[:], x[:])
            t2 = pool.tile([128, 512], F32)
            nc.scalar.mul(t2[:], t[:], 2.0)
            nc.sync.dma_start(y[:], t2[:])
    nc.finalize()
    im = [{"x": np.zeros((128, 512), np.float32)} for _ in range(NC_)]
    return nc, im


def bench_ns(inputs, pairs=40):
    import time
    import jax
    in_maps, NL, OL, XPAD, MCH_A = _prep(**inputs)
    nc = _build(NL, OL, XPAD, MCH_A)
    run_k, args_k = _make_runner(nc, in_maps)
    tnc, tim = _trivial_nc()
    run_t, args_t = _make_runner(tnc, tim)
    jax.block_until_ready(run_k(*args_k))
    jax.block_until_ready(run_t(*args_t))
    dk, dt = [], []
    for _ in range(pairs):
        t0 = time.perf_counter()
        jax.block_until_ready(run_t(*args_t))
        t1 = time.perf_counter()
        jax.block_until_ready(run_k(*args_k))
        t2 = time.perf_counter()
        dt.append(t1 - t0)
        dk.append(t2 - t1)
    dk, dt = np.array(dk), np.array(dt)
    est = np.median(dk) - np.median(dt)
    est_min = dk.min() - dt.min()
    return int(est * 1e9), int(est_min * 1e9)
